# revision 22
# baseline (speedup 1.0000x reference)
"""Trainium2 Bass kernel for nn_CausalAttention_84018150244353.

kernel(**inputs) takes the FULL unsharded inputs (as in reference
setup_inputs) and returns the full (2, 2048, 2048) float32 output.

Sharding: 8 NeuronCores = 2 batches x 4 head-groups (4 heads each).
Each core computes its head-group's QKV projections, causal ALiBi
attention (transposed layout: scores [j,i]) and its partial output
projection y^T = Wo_s^T @ out^T; the host sums the 4 head-group
partials per batch and adds bo (+ bv @ Wo, folded exactly).

vs baseline (579us -> ~392us measured):
  * bf16 matmul operands everywhere (PSUM accumulation stays fp32);
    halves DMA/SBUF, enables full-rate matmuls below 256 free columns.
  * q/k/v stay resident in SBUF between phases (no DRAM roundtrip).
  * mshift additions moved off the PE onto the vector engine (adds
    into PSUM scores); causal triangle mask added via a cheap bf16
    128-col ident matmul on the PE; ALiBi stays as the Exp bias;
    bq/bk folded into the phase-1 PSUM->SBUF copy bias; bv folded
    into the host-side output bias (bv @ Wo). PE runs only real
    matmuls + the softmax denominator reduction.
  * diagonal attention blocks restrict matmul/exp width to the
    causally-valid columns.
  * flat, globally software-pipelined attention schedule: the
    produce stream (QK -> DVE mshift add -> ACT exp -> p in SBUF)
    runs LOOK=5 j-chunks ahead of the consume stream (PV + den),
    crossing group and y-projection boundaries; per-group
    reciprocal/rescale DVE work is deferred past the next group's
    first adds so it never blocks the produce chain.
  * bf16 y^T output (halves the output write traffic).
"""
import math
import os
import sys
import time

sys.path.insert(0, "/opt/trn_rl_repo")

import numpy as np
import jax

jax.config.update("jax_compilation_cache_dir",
                  os.environ.get("JAX_NEFF_CACHE", "/tmp/jax_neff_cache"))
jax.config.update("jax_persistent_cache_min_compile_time_secs", 0.0)
jax.config.update("jax_persistent_cache_min_entry_size_bytes", -1)

from jax.sharding import Mesh, PartitionSpec
from jax.experimental.shard_map import shard_map

import concourse.bass as bass
import concourse.mybir as mybir
import concourse.tile as tile
from concourse import bacc
from concourse import bass2jax
from concourse.bass2jax import _bass_exec_p, install_neuronx_cc_hook

f32 = mybir.dt.float32
bf16 = mybir.dt.bfloat16
Exp = mybir.ActivationFunctionType.Exp
Identity = mybir.ActivationFunctionType.Identity
Copy = mybir.ActivationFunctionType.Copy

T = 2048
EMB = 2048
HG = 512          # columns per head group (4 heads x 128)
HD = 128
NH = 4            # heads per core
NQ = 4            # token quarters
QT = T // NQ      # 512
NE = EMB // 128   # 16 contraction chunks
NC_I = 4          # i-chunks of 512 queries
NJ = T // 128     # 16 key chunks


def build_program(reps: int = 1):
    nc = bacc.Bacc("TRN2", target_bir_lowering=False, debug=False,
                   enable_asserts=False, num_devices=8)

    xT_d = nc.dram_tensor("xT", [EMB, T], bf16, kind="ExternalInput")
    wq_d = nc.dram_tensor("wq", [EMB, HG], bf16, kind="ExternalInput")
    wk_d = nc.dram_tensor("wk", [EMB, HG], bf16, kind="ExternalInput")
    wv_d = nc.dram_tensor("wv", [EMB, HG], bf16, kind="ExternalInput")
    wo_d = nc.dram_tensor("wo", [HG, T], bf16, kind="ExternalInput")
    bqk_d = nc.dram_tensor("bqk", [128, 8], f32, kind="ExternalInput")
    alibi_d = nc.dram_tensor("alibi", [128, NH * NJ], f32, kind="ExternalInput")
    mshift_d = nc.dram_tensor("mshift", [1, NH * T], f32, kind="ExternalInput")
    trimask_d = nc.dram_tensor("trimask", [128, 128], bf16, kind="ExternalInput")
    ident_d = nc.dram_tensor("ident128", [128, 128], bf16, kind="ExternalInput")
    ones_d = nc.dram_tensor("ones128", [128, 128], bf16, kind="ExternalInput")
    yT_d = nc.dram_tensor("yT", [T, T], bf16, kind="ExternalOutput")

    with tile.TileContext(nc) as tc:
        with (
            tc.tile_pool(name="consts", bufs=1) as consts,
            tc.tile_pool(name="qkv", bufs=1) as qkv,
            tc.tile_pool(name="wop", bufs=1) as wop,
            tc.tile_pool(name="outfp", bufs=2) as outfp,
            tc.tile_pool(name="pp", bufs=7) as pp,
            tc.tile_pool(name="smallp", bufs=3) as smallp,
            tc.tile_pool(name="mshp", bufs=4) as mshp,
            tc.tile_pool(name="p3st", bufs=3) as p3st,
            tc.tile_pool(name="ps_s", bufs=4, space="PSUM") as ps_s,
            tc.tile_pool(name="ps_o", bufs=2, space="PSUM") as ps_o,
            tc.tile_pool(name="ps_d", bufs=2, space="PSUM") as ps_d,
        ):
            def body():
                # persistent QKV (bf16, SBUF-resident across both phases)
                qT_sb = qkv.tile([128, NH, T], bf16, name="qT_sb")
                kT_sb = qkv.tile([128, NH, T], bf16, name="kT_sb")
                v_sb = qkv.tile([128, NJ, HG], bf16, name="v_sb")

                # small constants
                bqk_sb = consts.tile([128, 8], f32, name="bqk_sb")
                alibi_sb = consts.tile([128, NH * NJ], f32, name="alibi_sb")
                trimask_sb = consts.tile([128, 128], bf16, name="trimask_sb")
                ident_sb = consts.tile([128, 128], bf16, name="ident_sb")
                ones_sb = consts.tile([128, 128], bf16, name="ones_sb")

                # ================= Phase 1: projections =================
                with (
                    tc.tile_pool(name="wslab", bufs=1) as wslab,
                    tc.tile_pool(name="xslab", bufs=2) as xslab,
                ):
                    xT_r = xT_d.ap().rearrange("(c p) t -> p c t", p=128)

                    # fine-grained first loads so the first projection
                    # group unblocks ASAP: wq cc0 and the leading x
                    # contraction chunks in small pieces
                    wq_sb = wslab.tile([128, NE, HG], bf16, name="wq_sb")
                    wk_sb = wslab.tile([128, NE, HG], bf16, name="wk_sb")
                    wv_sb = wslab.tile([128, NE, HG], bf16, name="wv_sb")
                    wq_r = wq_d.ap().rearrange("(c p) m -> p c m", p=128)
                    wk_r = wk_d.ap().rearrange("(c p) m -> p c m", p=128)
                    wv_r = wv_d.ap().rearrange("(c p) m -> p c m", p=128)

                    x_sb = xslab.tile([128, NE, QT], bf16, name="x_sb",
                                      tag="x_sb")

                    def wq_chunk(cc):
                        nc.sync.dma_start(
                            wq_sb[:, :, cc * 128:(cc + 1) * 128],
                            wq_r[:, :, cc * 128:(cc + 1) * 128])

                    nc.sync.dma_start(wq_sb[:, 0:8, 0:128],
                                      wq_r[:, 0:8, 0:128])
                    nc.sync.dma_start(x_sb[:, 0:2, :], xT_r[:, 0:2, 0:QT])
                    nc.sync.dma_start(wq_sb[:, 8:16, 0:128],
                                      wq_r[:, 8:16, 0:128])
                    nc.sync.dma_start(x_sb[:, 2:4, :], xT_r[:, 2:4, 0:QT])
                    nc.sync.dma_start(bqk_sb[:], bqk_d.ap())
                    for ec in range(1, 4):
                        nc.sync.dma_start(
                            x_sb[:, 4 * ec:4 * ec + 4, :],
                            xT_r[:, 4 * ec:4 * ec + 4, 0:QT])
                        wq_chunk(ec)
                    for cc in range(4):
                        nc.sync.dma_start(
                            wk_sb[:, :, cc * 128:(cc + 1) * 128],
                            wk_r[:, :, cc * 128:(cc + 1) * 128])
                    nc.sync.dma_start(wv_sb[:], wv_r)

                    # remaining small constants (needed at attention start)
                    nc.sync.dma_start(alibi_sb[:], alibi_d.ap())
                    nc.sync.dma_start(trimask_sb[:], trimask_d.ap())
                    nc.sync.dma_start(ident_sb[:], ident_d.ap())
                    nc.sync.dma_start(ones_sb[:], ones_d.ap())

                    for qt in range(NQ):
                        if qt > 0:
                            x_sb = xslab.tile([128, NE, QT], bf16, name="x_sb",
                                              tag="x_sb")
                            for ec in range(2):
                                nc.sync.dma_start(
                                    x_sb[:, 8 * ec:8 * ec + 8, :],
                                    xT_r[:, 8 * ec:8 * ec + 8,
                                         qt * QT:(qt + 1) * QT])

                        for pi, (w_sb, dst) in enumerate(
                                ((wq_sb, qT_sb), (wk_sb, kT_sb))):
                            for cc in range(4):
                                ps = ps_s.tile([128, 512], f32, name="p1acc",
                                               tag="s_ps")
                                for e in range(NE):
                                    nc.tensor.matmul(
                                        ps[:],
                                        w_sb[:, e, cc * 128:(cc + 1) * 128],
                                        x_sb[:, e, :],
                                        start=(e == 0), stop=(e == NE - 1))
                                nc.scalar.activation(
                                    dst[:, cc, qt * QT:(qt + 1) * QT], ps[:],
                                    Identity,
                                    bias=bqk_sb[:, 4 * pi + cc:4 * pi + cc + 1])

                        for tb in range(4):
                            ps = ps_s.tile([128, 512], f32, name="p1acc",
                                           tag="s_ps")
                            for e in range(NE):
                                nc.tensor.matmul(
                                    ps[:],
                                    x_sb[:, e, tb * 128:(tb + 1) * 128],
                                    wv_sb[:, e, :],
                                    start=(e == 0), stop=(e == NE - 1))
                            nc.scalar.copy(v_sb[:, qt * 4 + tb, :], ps[:])

                # ========= Phase 2+3: attention + inlined projection =========
                # wo: no deps, loads during attention ramp-up
                wo_sb = wop.tile([128, NH, T], bf16, name="wo_sb")
                nc.sync.dma_start(
                    wo_sb[:],
                    wo_d.ap().rearrange("(h p) o -> p h o", p=128))

                # Flat, globally software-pipelined schedule. The
                # "produce" stream (QK matmul -> DVE mshift/mask add ->
                # ACT exp -> p tile in SBUF) runs LOOK j-chunks ahead of
                # the "consume" stream (PV + den PSUM accumulation),
                # crossing (c,h)-group and y-projection boundaries so no
                # engine drains at a boundary.
                LOOK = 5
                groups = [(c, h) for c in range(NC_I) for h in range(NH)]
                sc_items = [(gi, jc) for gi, (c, h) in enumerate(groups)
                            for jc in range(4 * c + 4)]
                flat = []
                for gi, (c, h) in enumerate(groups):
                    for jc in range(4 * c + 4):
                        flat.append(("jc", gi, jc))
                    flat.append(("fin", gi, 0))
                    if h == NH - 1:
                        flat.append(("y", gi, c))

                msh_tiles = {}
                p_tiles = {}     # (gi, jc) -> p tile
                gstate = {}      # gi -> (outp, den)
                outf_tiles = {}  # c -> outf tile
                pending_fin = []  # deferred (rcp, mul) DVE work: (outp, den, c, h)

                def fetch_msh(gi):
                    if gi in msh_tiles or gi >= len(groups):
                        return
                    c, h = groups[gi]
                    msh = mshp.tile([128, 512], f32, name="msh_sb",
                                    tag="msh_sb")
                    nc.sync.dma_start(
                        msh[:],
                        mshift_d.ap()[0:1, h * T + c * 512:
                                      h * T + (c + 1) * 512]
                        .to_broadcast((128, 512)))
                    msh_tiles[gi] = msh

                def flush_fin():
                    # one DVE op per call so the per-group softmax epilogue
                    # never bursts the vector engine (which runs at parity
                    # with the PE during attention)
                    kind2, args = pending_fin.pop(0)
                    with nc.allow_low_precision(
                            reason="elementwise reciprocal/rescale"):
                        if kind2 == "rcp":
                            rcp, den = args
                            nc.vector.reciprocal(rcp[:], den[:])
                        else:
                            c, h, outp, rcp = args
                            nc.vector.tensor_mul(
                                outf_tiles[c][:, h, :], outp[:], rcp[:])

                def emit_produce(idx):
                    gi, jc = sc_items[idx]
                    c, h = groups[gi]
                    if jc == 0:
                        fetch_msh(gi)
                        fetch_msh(gi + 1)
                    msh = msh_tiles[gi]
                    d = jc - 4 * c
                    lo = 128 * max(d, 0)
                    s = ps_s.tile([128, 512], f32, name="s_ps", tag="s_ps")
                    nc.tensor.matmul(
                        s[:, lo:512],
                        kT_sb[:, h, jc * 128:(jc + 1) * 128],
                        qT_sb[:, h, c * 512 + lo:(c + 1) * 512],
                        start=True, stop=(d < 0),
                        skip_group_check=True)
                    if d >= 0:
                        # causal triangle mask added on the PE (cheap bf16
                        # 128-col matmul) instead of a DVE add
                        nc.tensor.matmul(
                            s[:, lo:lo + 128], ident_sb[:], trimask_sb[:],
                            start=False, stop=True, skip_group_check=True)
                    nc.vector.tensor_add(
                        s[:, lo:512], s[:, lo:512], msh[:, lo:512])
                    if pending_fin:
                        flush_fin()
                    p = pp.tile([128, 512], bf16, name="p_sb", tag="p_sb")
                    nc.scalar.activation(
                        p[:, lo:512], s[:, lo:512], Exp,
                        bias=alibi_sb[:, h * NJ + jc:h * NJ + jc + 1])
                    p_tiles[(gi, jc)] = p

                fetch_msh(0)
                si = 0
                ci = 0  # count of consumed jc items
                for kind, gi, arg in flat:
                    if kind == "jc":
                        while si < len(sc_items) and si <= ci + LOOK:
                            emit_produce(si)
                            si += 1
                        jc = arg
                        c, h = groups[gi]
                        nj = 4 * c + 4
                        if jc == 0:
                            outp = ps_o.tile([128, 512], f32, name="out_ps",
                                             tag="out_ps")
                            den = ps_d.tile([128, 512], f32, name="den_ps",
                                            tag="den_ps")
                            gstate[gi] = (outp, den)
                        outp, den = gstate[gi]
                        lo = 128 * max(jc - 4 * c, 0)
                        p = p_tiles.pop((gi, jc))
                        nc.tensor.matmul(
                            outp[:, lo:512],
                            v_sb[:, jc, h * 128:(h + 1) * 128],
                            p[:, lo:512],
                            start=(jc == 0), stop=(jc == nj - 1),
                            skip_group_check=True)
                        nc.tensor.matmul(
                            den[:, lo:512], ones_sb[:], p[:, lo:512],
                            start=(jc == 0), stop=(jc == nj - 1),
                            skip_group_check=True)
                        ci += 1
                    elif kind == "fin":
                        c, h = groups[gi]
                        if h == 0:
                            outf_tiles[c] = outfp.tile(
                                [128, NH, 512], bf16, name="outf_c",
                                tag="outf_c")
                        outp, den = gstate.pop(gi)
                        rcp = smallp.tile([128, 512], f32, name="rcp",
                                          tag="rcp")
                        pending_fin.append(("rcp", (rcp, den)))
                        pending_fin.append(("mul", (c, h, outp, rcp)))
                    else:  # y projection for i-chunk c
                        c = arg
                        while pending_fin:
                            flush_fin()
                        outf_c = outf_tiles.pop(c)
                        for oc in range(16):
                            yp = ps_s.tile([128, 512], f32, name="y_ps",
                                           tag="s_ps")
                            for h in range(NH):
                                nc.tensor.matmul(
                                    yp[:],
                                    wo_sb[:, h, oc * 128:(oc + 1) * 128],
                                    outf_c[:, h, :],
                                    start=(h == 0), stop=(h == 3))
                            ys = p3st.tile([128, 512], bf16, name="y_sb",
                                           tag="y_sb")
                            nc.scalar.copy(ys[:], yp[:])
                            nc.sync.dma_start(
                                yT_d.ap()[oc * 128:(oc + 1) * 128,
                                          c * 512:(c + 1) * 512],
                                ys[:])

            if reps == 1:
                body()
            else:
                with tc.For_i(0, reps, 1):
                    body()

    nc.compile()
    return nc


def make_host_inputs(x, Wq, bq, Wk, bk, Wv, bv, Wo, bo):
    """Shard full inputs into 8 per-core input maps."""
    from ml_dtypes import bfloat16

    x = np.asarray(x, np.float32)
    Wq = np.asarray(Wq, np.float32); bq = np.asarray(bq, np.float32)
    Wk = np.asarray(Wk, np.float32); bk = np.asarray(bk, np.float32)
    Wv = np.asarray(Wv, np.float32)
    Wo = np.asarray(Wo, np.float32)

    NUM_HEAD = 16
    start = 2 ** (-2 ** (-(math.log2(NUM_HEAD) - 3)))
    slopes = np.array([start * start ** i for i in range(NUM_HEAD)], np.float32)

    sc = np.float32(1.0 / math.sqrt(HD))
    jl = np.arange(128, dtype=np.float32)
    jcs = np.arange(NJ, dtype=np.float32)
    key_idx = (jcs[None, :] * 128 + jl[:, None])  # [128, NJ]

    trimask = np.where(jl[:, None] > jl[None, :],
                       np.float32(-1e10), np.float32(0.0)).astype(bfloat16)
    ident128 = np.eye(128, dtype=np.float32).astype(bfloat16)
    ones128 = np.ones((128, 128), bfloat16)
    i_idx = np.arange(T, dtype=np.float32)

    in_maps = []
    for core in range(8):
        b, hg = core // 4, core % 4
        cols = slice(hg * HG, (hg + 1) * HG)
        heads = slopes[hg * NH:(hg + 1) * NH]
        alibi = np.empty((128, NH * NJ), np.float32)
        mshift = np.empty((1, NH * T), np.float32)
        for h in range(NH):
            alibi[:, h * NJ:(h + 1) * NJ] = -heads[h] * (T - 1 - key_idx)
            mshift[0, h * T:(h + 1) * T] = heads[h] * (T - 1 - i_idx)
        bqk = np.empty((128, 8), np.float32)
        bqk[:, 0:4] = (bq[cols] * sc).reshape(4, 128).T
        bqk[:, 4:8] = bk[cols].reshape(4, 128).T
        in_maps.append({
            "xT": np.ascontiguousarray(x[b].T).astype(bfloat16),
            "wq": (np.ascontiguousarray(Wq[:, cols]) * sc).astype(bfloat16),
            "wk": np.ascontiguousarray(Wk[:, cols]).astype(bfloat16),
            "wv": np.ascontiguousarray(Wv[:, cols]).astype(bfloat16),
            "wo": np.ascontiguousarray(Wo[cols, :]).astype(bfloat16),
            "bqk": bqk,
            "alibi": alibi,
            "mshift": mshift,
            "trimask": trimask,
            "ident128": ident128,
            "ones128": ones128,
        })
    return in_maps


def assemble_output(results, bo):
    """results: list of 8 per-core dicts with 'yT'. Returns (2, T, EMB).

    bo here is the effective output bias (bo + bv @ Wo)."""
    bo = np.asarray(bo, np.float32)
    out = np.empty((2, T, EMB), np.float32)
    for b in range(2):
        acc = results[b * 4 + 0]["yT"].astype(np.float32)
        for hg in range(1, 4):
            acc += results[b * 4 + hg]["yT"].astype(np.float32)
        out[b] = acc.T + bo
    return out


class SpmdRunner:
    def __init__(self, nc, n_cores: int):
        install_neuronx_cc_hook()
        self.nc = nc
        self.n_cores = n_cores
        assert nc.dbg_addr is None or not nc.dbg_callbacks
        partition_name = (
            nc.partition_id_tensor.name if nc.partition_id_tensor else None
        )
        in_names, out_names, out_avals = [], [], []
        for alloc in nc.m.functions[0].allocations:
            if not isinstance(alloc, mybir.MemoryLocationSet):
                continue
            name = alloc.memorylocations[0].name
            if alloc.kind == "ExternalInput":
                if name != partition_name:
                    in_names.append(name)
            elif alloc.kind == "ExternalOutput":
                shape = tuple(alloc.tensor_shape)
                dtype = mybir.dt.np(alloc.dtype)
                out_names.append(name)
                out_avals.append(jax.core.ShapedArray(shape, dtype))
        self.in_names = list(in_names)
        self.out_names = out_names
        self.out_avals = out_avals
        n_params = len(self.in_names)
        all_in_names = list(in_names) + list(out_names)
        if partition_name is not None:
            all_in_names.append(partition_name)
        self.partition_name = partition_name

        def _body(*args):
            operands = list(args)
            if partition_name is not None:
                operands.append(bass2jax.partition_id_tensor())
            outs = _bass_exec_p.bind(
                *operands,
                out_avals=tuple(out_avals),
                in_names=tuple(all_in_names),
                out_names=tuple(out_names),
                lowering_input_output_aliases=(),
                sim_require_finite=True,
                sim_require_nnan=True,
                nc=nc,
            )
            return tuple(outs)

        devices = jax.devices()[:n_cores]
        assert len(devices) == n_cores
        self.mesh = Mesh(np.asarray(devices), ("core",))
        n_outs = len(out_names)
        in_specs = (PartitionSpec("core"),) * (n_params + n_outs)
        out_specs = (PartitionSpec("core"),) * n_outs
        self.fn = jax.jit(
            shard_map(_body, mesh=self.mesh, in_specs=in_specs,
                      out_specs=out_specs, check_rep=False),
            keep_unused=True,
        )
        self.dev_args = None

    def set_inputs(self, in_maps: list[dict]):
        """device_put concatenated per-core inputs + zero output buffers."""
        n = self.n_cores
        assert len(in_maps) == n
        concat_in = [
            np.concatenate([np.asarray(in_maps[c][name]) for c in range(n)], axis=0)
            for name in self.in_names
        ]
        concat_zeros = [
            np.zeros((n * a.shape[0], *a.shape[1:]), a.dtype) for a in self.out_avals
        ]
        sharding = jax.sharding.NamedSharding(self.mesh, PartitionSpec("core"))
        self.dev_args = [jax.device_put(a, sharding) for a in concat_in + concat_zeros]

    def run(self):
        outs = self.fn(*self.dev_args)
        jax.block_until_ready(outs)
        return outs

    def results(self, outs) -> list[dict]:
        n = self.n_cores
        return [
            {
                name: np.asarray(outs[i]).reshape(n, *self.out_avals[i].shape)[c]
                for i, name in enumerate(self.out_names)
            }
            for c in range(n)
        ]

    def time_execs(self, iters: int = 10, warmup: int = 2):
        for _ in range(warmup):
            self.run()
        t0 = time.perf_counter()
        for _ in range(iters):
            outs = self.fn(*self.dev_args)
        jax.block_until_ready(outs)
        t1 = time.perf_counter()
        return (t1 - t0) / iters


_RUNNER = None


def _get_runner():
    global _RUNNER
    if _RUNNER is None:
        nc = build_program(reps=1)
        _RUNNER = SpmdRunner(nc, 8)
    return _RUNNER


def kernel(x, Wq, bq, Wk, bk, Wv, bv, Wo, bo):
    r = _get_runner()
    in_maps = make_host_inputs(x, Wq, bq, Wk, bk, Wv, bv, Wo, bo)
    r.set_inputs(in_maps)
    outs = r.run()
    res = r.results(outs)
    bo_eff = (np.asarray(bo, np.float32)
              + np.asarray(bv, np.float32) @ np.asarray(Wo, np.float32))
    return assemble_output(res, bo_eff)
